# revision 9
# baseline (speedup 1.0000x reference)
"""ChebyKAN projector (2-layer Chebyshev-KAN MLP + sync-BN) on 8 TRN2
NeuronCores.

Math (per reference):
    h   = x @ w1^T + b1 + s1 * sum_d T_d(tanh(x)) @ c1[:,:,d]
    hbn = BN(h)  (batch stats over the full 8192-row batch)
    out = hbn @ w2^T + b2 + s2 * sum_d T_d(tanh(hbn)) @ c2[:,:,d]

T_0 = 1 is folded into the bias on the host (column-sum of c1[:,:,0]),
and the per-output scaler s is folded into the coefficient matrices, so
each layer is a single GEMM over a concatenation of 5 streams
[x, T1, T2, T3, T4] with contraction dim 5*2048 = 10240.

Sharding: data-parallel over the batch (1024 rows/core), activations
kept feature-on-partition (transposed) so BatchNorm reduces along the
free axis; BN batch stats sync'd with one 16 KB AllReduce.

Matmuls run in bf16 (fp32 PSUM accumulation).
"""

import numpy as np
import ml_dtypes

import concourse.bacc as bacc
import concourse.mybir as mybir
import concourse.tile as tile
from concourse.bass_utils import run_bass_kernel_spmd

N_CORES = 8
B, IN, H, OUT = 8192, 2048, 2048, 512
BC = B // N_CORES          # batch rows per core (1024)
DEG = 4
BN_EPS = 1e-5

NS = 5                     # streams: x, T1, T2, T3, T4

F32 = mybir.dt.float32
BF16 = mybir.dt.bfloat16
AF = mybir.ActivationFunctionType
OP = mybir.AluOpType

_CACHED_NC = None


def _emit_cheby_streams(nc, sc, src_ap, streams, j):
    """Emit tanh + Chebyshev recurrence for one [128, 512] chunk.

    src_ap: bf16 [128, 512] input chunk (x or BN'd h).
    streams: list of 4 bf16 [128, 16*1024] tiles (T1..T4 destinations).
    j: chunk index (column offset j*512).
    """
    c0 = j * 512
    xf = sc.tile([128, 512], F32, tag="xf", name=f"xf_{j}")
    nc.scalar.activation(xf[:], src_ap, AF.Tanh)
    # T1 = tanh(x)
    nc.scalar.copy(streams[0][:, c0:c0 + 512], xf[:])
    # T2 = 2*x~^2 - 1
    pf = sc.tile([128, 512], F32, tag="pf", name=f"pf_{j}")
    nc.vector.tensor_mul(pf[:], xf[:], xf[:])
    t2 = sc.tile([128, 512], F32, tag="t2", name=f"t2_{j}")
    nc.vector.tensor_scalar(t2[:], pf[:], 2.0, -1.0, OP.mult, OP.add)
    nc.scalar.copy(streams[1][:, c0:c0 + 512], t2[:])
    # T3 = 2*x~*T2 - x~
    p3 = sc.tile([128, 512], F32, tag="pf", name=f"p3_{j}")
    nc.vector.scalar_tensor_tensor(p3[:], xf[:], 2.0, t2[:], OP.mult, OP.mult)
    t3 = sc.tile([128, 512], F32, tag="t3", name=f"t3_{j}")
    nc.vector.tensor_sub(t3[:], p3[:], xf[:])
    nc.scalar.copy(streams[2][:, c0:c0 + 512], t3[:])
    # T4 = 2*x~*T3 - T2  (written straight to bf16)
    p4 = sc.tile([128, 512], F32, tag="pf", name=f"p4_{j}")
    nc.vector.scalar_tensor_tensor(p4[:], xf[:], 2.0, t3[:], OP.mult, OP.mult)
    nc.vector.tensor_sub(streams[3][:, c0:c0 + 512], p4[:], t2[:])


def _build(n_cores=N_CORES, in_=IN, h_=H, out_=OUT, bc=BC, b_total=B):
    KT = in_ // 128        # contraction k-tiles per stream (layer 1)
    KT2 = h_ // 128        # contraction k-tiles per stream (layer 2)
    MT1 = h_ // 128        # layer-1 output feature tiles
    MT2 = out_ // 128      # layer-2 output feature tiles
    NCHUNK = bc // 512     # batch n-chunks of 512

    nc = bacc.Bacc("TRN2", target_bir_lowering=False, debug=False,
                   num_devices=n_cores)

    xt = nc.dram_tensor("xt", [in_, bc], BF16, kind="ExternalInput").ap()
    w1 = nc.dram_tensor("w1", [NS * in_, h_], BF16, kind="ExternalInput").ap()
    b1 = nc.dram_tensor("b1", [128, MT1], F32, kind="ExternalInput").ap()
    gam = nc.dram_tensor("gam", [128, MT1], F32, kind="ExternalInput").ap()
    bet = nc.dram_tensor("bet", [128, MT1], F32, kind="ExternalInput").ap()
    w2 = nc.dram_tensor("w2", [NS * h_, out_], BF16, kind="ExternalInput").ap()
    b2 = nc.dram_tensor("b2", [128, MT2], F32, kind="ExternalInput").ap()
    out_t = nc.dram_tensor("out_t", [out_, bc], F32, kind="ExternalOutput").ap()

    assert NCHUNK == 2, "stream column layout assumes bc == 1024"

    with tile.TileContext(nc) as tc:
        with (
            tc.tile_pool(name="sp", bufs=1) as sp,       # persistent streams
            tc.tile_pool(name="sc", bufs=2) as sc,       # elementwise scratch
            tc.tile_pool(name="wp", bufs=6) as wp,       # weight staging
            tc.tile_pool(name="ps", bufs=8, space="PSUM") as ps,
            tc.tile_pool(name="dp", bufs=1, space="DRAM") as dp,
        ):
            # small constants
            b1s = sp.tile([128, MT1], F32, name="b1s")
            nc.sync.dma_start(b1s[:], b1)
            gams = sp.tile([128, MT1], F32, name="gams")
            nc.sync.dma_start(gams[:], gam)
            bets = sp.tile([128, MT1], F32, name="bets")
            nc.sync.dma_start(bets[:], bet)
            b2s = sp.tile([128, MT2], F32, name="b2s")
            nc.sync.dma_start(b2s[:], b2)

            # ---- phase 1: layer-1 Chebyshev streams (T1..T4) ----
            st1 = [sp.tile([128, KT * 1024], BF16, tag=f"s{d}", name=f"l1s{d}")
                   for d in range(1, 5)]
            for j in range(KT * NCHUNK):
                k, n = j // 2, j % 2
                xbf = sc.tile([128, 512], BF16, tag="xbf", name=f"xbf_{j}")
                nc.sync.dma_start(
                    xbf[:], xt[k * 128:(k + 1) * 128, n * 512:(n + 1) * 512])
                _emit_cheby_streams(nc, sc, xbf[:], st1, j)

            # layer-1 h accumulator / later the BN'd layer-2 base stream
            hh = sp.tile([128, MT1 * 1024], BF16, name="hh")
            s1p = [sp.tile([128, MT1], F32, name=f"s1p{n}") for n in range(2)]
            s2p = [sp.tile([128, MT1], F32, name=f"s2p{n}") for n in range(2)]

            # ---- phase 2: layer-1 GEMM, 8 groups of 2 m-tiles ----
            for g in range(MT1 // 2):
                acc = [ps.tile([128, 512], F32, tag="acc",
                               name=f"acc1_{g}_{i}") for i in range(4)]
                for k in range(KT):
                    xr = wp.tile([128, 1024], BF16, tag="xr", name=f"xr_{g}_{k}")
                    nc.sync.dma_start(xr[:], xt[k * 128:(k + 1) * 128, :])
                    for s in range(NS):
                        wt = wp.tile([128, 256], BF16, tag="w",
                                     name=f"w1_{g}_{k}_{s}")
                        nc.sync.dma_start(
                            wt[:], w1[s * in_ + k * 128: s * in_ + (k + 1) * 128,
                                      g * 256:(g + 1) * 256])
                        for mi in range(2):
                            for n in range(2):
                                if s == 0:
                                    rhs = xr[:, n * 512:(n + 1) * 512]
                                else:
                                    c0 = k * 1024 + n * 512
                                    rhs = st1[s - 1][:, c0:c0 + 512]
                                nc.tensor.matmul(
                                    acc[mi * 2 + n][:],
                                    wt[:, mi * 128:(mi + 1) * 128], rhs,
                                    start=(k == 0 and s == 0),
                                    stop=(k == KT - 1 and s == NS - 1))
                # finalize: bias add, cast to bf16 h, BN partial sums
                for mi in range(2):
                    m = g * 2 + mi
                    for n in range(2):
                        hs = hh[:, m * 1024 + n * 512: m * 1024 + (n + 1) * 512]
                        nc.scalar.activation(
                            hs, acc[mi * 2 + n][:], AF.Identity,
                            bias=b1s[:, m:m + 1], scale=1.0,
                            accum_out=s1p[n][:, m:m + 1])
                        sq = sc.tile([128, 512], BF16, tag="sq",
                                     name=f"sq_{g}_{mi}_{n}")
                        nc.scalar.activation(
                            sq[:], acc[mi * 2 + n][:], AF.Square,
                            bias=b1s[:, m:m + 1], scale=1.0,
                            accum_out=s2p[n][:, m:m + 1])

            # ---- phase 3: sync-BN stats (AllReduce) + coefficients ----
            st = sp.tile([128, 2 * MT1], F32, name="st")
            nc.vector.tensor_add(st[:, 0:MT1], s1p[0][:], s1p[1][:])
            nc.vector.tensor_add(st[:, MT1:2 * MT1], s2p[0][:], s2p[1][:])
            bnc_in = dp.tile([128, 2 * MT1], F32, name="bnc_in")
            bnc_out = dp.tile([128, 2 * MT1], F32, addr_space="Shared",
                              name="bnc_out")
            nc.sync.dma_start(bnc_in[:], st[:])
            nc.gpsimd.collective_compute(
                "AllReduce", OP.add,
                ins=[bnc_in.opt()], outs=[bnc_out.opt()],
                replica_groups=[list(range(n_cores))])
            sto = sp.tile([128, 2 * MT1], F32, name="sto")
            nc.sync.dma_start(sto[:], bnc_out[:])

            meanv = sp.tile([128, MT1], F32, name="meanv")
            nc.vector.tensor_scalar_mul(meanv[:], sto[:, 0:MT1], 1.0 / b_total)
            e2 = sp.tile([128, MT1], F32, name="e2")
            nc.vector.tensor_scalar_mul(e2[:], sto[:, MT1:2 * MT1], 1.0 / b_total)
            mm2 = sp.tile([128, MT1], F32, name="mm2")
            nc.vector.tensor_mul(mm2[:], meanv[:], meanv[:])
            varv = sp.tile([128, MT1], F32, name="varv")
            nc.vector.tensor_sub(varv[:], e2[:], mm2[:])
            nc.vector.tensor_scalar_add(varv[:], varv[:], BN_EPS)
            invv = sp.tile([128, MT1], F32, name="invv")
            nc.vector.reciprocal(invv[:], varv[:])
            istd = sp.tile([128, MT1], F32, name="istd")
            nc.scalar.sqrt(istd[:], invv[:])
            scl = sp.tile([128, MT1], F32, name="scl")
            nc.vector.tensor_mul(scl[:], gams[:], istd[:])
            msc = sp.tile([128, MT1], F32, name="msc")
            nc.vector.tensor_mul(msc[:], meanv[:], scl[:])
            shf = sp.tile([128, MT1], F32, name="shf")
            nc.vector.tensor_sub(shf[:], bets[:], msc[:])

            # ---- phase 4: BN apply (in place) + layer-2 streams ----
            st2 = [sp.tile([128, KT2 * 1024], BF16, tag=f"s{d}", name=f"l2s{d}")
                   for d in range(1, 5)]
            for j in range(MT1 * NCHUNK):
                m = j // 2
                c0 = j * 512
                hs = hh[:, c0:c0 + 512]
                nc.vector.tensor_scalar(
                    hs, hs, scl[:, m:m + 1], shf[:, m:m + 1], OP.mult, OP.add)
                _emit_cheby_streams(nc, sc, hs, st2, j)

            # ---- phase 5: layer-2 GEMM, 2 groups of 2 m-tiles ----
            for g in range(MT2 // 2):
                acc = [ps.tile([128, 512], F32, tag="acc",
                               name=f"acc2_{g}_{i}") for i in range(4)]
                for k in range(KT2):
                    for s in range(NS):
                        wt = wp.tile([128, 256], BF16, tag="w",
                                     name=f"w2_{g}_{k}_{s}")
                        nc.sync.dma_start(
                            wt[:], w2[s * h_ + k * 128: s * h_ + (k + 1) * 128,
                                      g * 256:(g + 1) * 256])
                        for mi in range(2):
                            for n in range(2):
                                c0 = k * 1024 + n * 512
                                if s == 0:
                                    rhs = hh[:, c0:c0 + 512]
                                else:
                                    rhs = st2[s - 1][:, c0:c0 + 512]
                                nc.tensor.matmul(
                                    acc[mi * 2 + n][:],
                                    wt[:, mi * 128:(mi + 1) * 128], rhs,
                                    start=(k == 0 and s == 0),
                                    stop=(k == KT - 1 and s == NS - 1))
                for mi in range(2):
                    m = g * 2 + mi
                    osb = sc.tile([128, 1024], F32, tag="osb",
                                  name=f"osb_{g}_{mi}")
                    for n in range(2):
                        nc.scalar.activation(
                            osb[:, n * 512:(n + 1) * 512], acc[mi * 2 + n][:],
                            AF.Identity, bias=b2s[:, m:m + 1], scale=1.0)
                    nc.sync.dma_start(out_t[m * 128:(m + 1) * 128, :], osb[:])

    nc.compile()
    return nc


def _get_nc():
    global _CACHED_NC
    if _CACHED_NC is None:
        _CACHED_NC = _build()
    return _CACHED_NC


def kernel(x, w1_base, b1_base, c1, s1, bn_gamma, bn_beta,
           w2_base, b2_base, c2, s2, _trace=False, **_trace_kwargs):
    x = np.asarray(x, np.float32)
    bf = ml_dtypes.bfloat16

    # Fold T0=1 into the bias; fold the scaler into the coefficients.
    w1p = np.empty((NS * IN, H), bf)
    w1p[0:IN] = w1_base.T.astype(bf)
    for d in range(1, DEG + 1):
        w1p[d * IN:(d + 1) * IN] = (c1[:, :, d] * s1[None, :]).astype(bf)
    b1e = (b1_base + s1 * c1[:, :, 0].sum(axis=0)).astype(np.float32)

    w2p = np.empty((NS * H, OUT), bf)
    w2p[0:H] = w2_base.T.astype(bf)
    for d in range(1, DEG + 1):
        w2p[d * H:(d + 1) * H] = (c2[:, :, d] * s2[None, :]).astype(bf)
    b2e = (b2_base + s2 * c2[:, :, 0].sum(axis=0)).astype(np.float32)

    def fold_pm(v, mt):
        return np.ascontiguousarray(
            np.asarray(v, np.float32).reshape(mt, 128).T)

    mt1, mt2 = H // 128, OUT // 128
    common = {
        "w1": w1p, "b1": fold_pm(b1e, mt1),
        "gam": fold_pm(bn_gamma, mt1), "bet": fold_pm(bn_beta, mt1),
        "w2": w2p, "b2": fold_pm(b2e, mt2),
    }
    in_maps = []
    for c in range(N_CORES):
        xs = np.ascontiguousarray(x[c * BC:(c + 1) * BC].T.astype(bf))
        in_maps.append({"xt": xs, **common})

    nc = _get_nc()
    res = run_bass_kernel_spmd(nc, in_maps, core_ids=list(range(N_CORES)),
                               trace=_trace, **_trace_kwargs)
    out = np.concatenate(
        [res.results[c]["out_t"].T for c in range(N_CORES)], axis=0)
    if _trace:
        return np.ascontiguousarray(out, dtype=np.float32), res
    return np.ascontiguousarray(out, dtype=np.float32)


# revision 34
# speedup vs baseline: 53.9419x; 53.9419x over previous
"""ChebyKAN projector (2-layer Chebyshev-KAN MLP + sync-BN) on 8 TRN2
NeuronCores.

Math (per reference):
    h   = x @ w1^T + b1 + s1 * sum_d T_d(tanh(x)) @ c1[:,:,d]
    hbn = BN(h)  (batch stats over the full 8192-row batch)
    out = hbn @ w2^T + b2 + s2 * sum_d T_d(tanh(hbn)) @ c2[:,:,d]

T_0 = 1 is folded into the bias on the host (column-sum of c1[:,:,0]),
and the per-output scaler s is folded into the coefficient matrices, so
each layer is a single GEMM over a concatenation of 5 streams
[x, T1, T2, T3, T4] with contraction dim 5*2048 = 10240.

Sharding: data-parallel over the batch (1024 rows/core), activations
kept feature-on-partition (transposed) so BatchNorm reduces along the
free axis; BN batch stats sync'd with one 16 KB AllReduce.

Matmuls run in bf16 (fp32 PSUM accumulation).
"""

import numpy as np
import ml_dtypes

import concourse.bacc as bacc
import concourse.mybir as mybir
import concourse.tile as tile
from concourse.bass_utils import run_bass_kernel_spmd

N_CORES = 8
B, IN, H, OUT = 8192, 2048, 2048, 512
BC = B // N_CORES          # batch rows per core (1024)
DEG = 4
BN_EPS = 1e-5

NS = 5                     # streams: x, T1, T2, T3, T4

F32 = mybir.dt.float32
BF16 = mybir.dt.bfloat16
AF = mybir.ActivationFunctionType
OP = mybir.AluOpType

_CACHED_NC = None


def _emit_cheby_streams(nc, sc, src_ap, streams, j):
    """Emit tanh + Chebyshev recurrence for one [128, 512] chunk.

    src_ap: bf16 [128, 512] input chunk (x or BN'd h).
    streams: list of 4 bf16 [128, 16*1024] tiles (T1..T4 destinations).
    j: chunk index (column offset j*512).
    """
    c0 = j * 512
    xf = sc.tile([128, 512], F32, tag="xf", name=f"xf_{j}")
    nc.scalar.activation(xf[:], src_ap, AF.Tanh)
    # T1 = tanh(x)
    nc.scalar.copy(streams[0][:, c0:c0 + 512], xf[:])
    # T2 = 2*x~^2 - 1
    pf = sc.tile([128, 512], F32, tag="pf", name=f"pf_{j}")
    nc.vector.tensor_mul(pf[:], xf[:], xf[:])
    t2 = sc.tile([128, 512], F32, tag="t2", name=f"t2_{j}")
    nc.vector.tensor_scalar(t2[:], pf[:], 2.0, -1.0, OP.mult, OP.add)
    nc.scalar.copy(streams[1][:, c0:c0 + 512], t2[:])
    # T3 = 2*x~*T2 - x~
    p3 = sc.tile([128, 512], F32, tag="pf", name=f"p3_{j}")
    nc.vector.scalar_tensor_tensor(p3[:], xf[:], 2.0, t2[:], OP.mult, OP.mult)
    t3 = sc.tile([128, 512], F32, tag="t3", name=f"t3_{j}")
    nc.vector.tensor_sub(t3[:], p3[:], xf[:])
    nc.scalar.copy(streams[2][:, c0:c0 + 512], t3[:])
    # T4 = 2*x~*T3 - T2  (written straight to bf16)
    p4 = sc.tile([128, 512], F32, tag="pf", name=f"p4_{j}")
    nc.vector.scalar_tensor_tensor(p4[:], xf[:], 2.0, t3[:], OP.mult, OP.mult)
    nc.vector.tensor_sub(streams[3][:, c0:c0 + 512], p4[:], t2[:])


def _build(n_cores=N_CORES, in_=IN, h_=H, out_=OUT, bc=BC, b_total=B,
           loop_k=1, no_cc=False):
    KT = in_ // 128        # contraction k-tiles per stream (layer 1)
    KT2 = h_ // 128        # contraction k-tiles per stream (layer 2)
    MT1 = h_ // 128        # layer-1 output feature tiles
    MT2 = out_ // 128      # layer-2 output feature tiles
    NCHUNK = bc // 512     # batch n-chunks of 512

    nc = bacc.Bacc("TRN2", target_bir_lowering=False, debug=False,
                   num_devices=n_cores)

    xt = nc.dram_tensor("xt", [in_, bc], BF16, kind="ExternalInput").ap()
    w1 = nc.dram_tensor("w1", [NS * in_, h_], BF16, kind="ExternalInput").ap()
    b1 = nc.dram_tensor("b1", [128, MT1], F32, kind="ExternalInput").ap()
    gam = nc.dram_tensor("gam", [128, MT1], F32, kind="ExternalInput").ap()
    bet = nc.dram_tensor("bet", [128, MT1], F32, kind="ExternalInput").ap()
    w2 = nc.dram_tensor("w2", [NS * h_, out_], BF16, kind="ExternalInput").ap()
    b2 = nc.dram_tensor("b2", [128, MT2], F32, kind="ExternalInput").ap()
    out_t = nc.dram_tensor("out_t", [out_, bc], F32, kind="ExternalOutput").ap()

    assert NCHUNK == 2, "stream column layout assumes bc == 1024"

    with tile.TileContext(nc) as tc:
        with (
            tc.tile_pool(name="sp", bufs=1) as sp,       # persistent streams
            tc.tile_pool(name="sc", bufs=2) as sc,       # elementwise scratch
            tc.tile_pool(name="wp", bufs=8) as wp,       # weight staging
            tc.tile_pool(name="ps", bufs=8, space="PSUM") as ps,
            tc.tile_pool(name="dp", bufs=1, space="DRAM") as dp,
        ):
            # small constants
            b1s = sp.tile([128, MT1], F32, name="b1s")
            nc.sync.dma_start(b1s[:], b1)
            gams = sp.tile([128, MT1], F32, name="gams")
            nc.sync.dma_start(gams[:], gam)
            bets = sp.tile([128, MT1], F32, name="bets")
            nc.sync.dma_start(bets[:], bet)
            b2s = sp.tile([128, MT2], F32, name="b2s")
            nc.sync.dma_start(b2s[:], b2)

            if loop_k > 1:
                # timing-only mode: device-side repeat of the whole body;
                # the collective is replaced by a DMA copy (collectives
                # cannot live inside control flow).
                loop_cm = tc.For_i(0, loop_k, 1, hint_engines=(
                    mybir.EngineType.PE, mybir.EngineType.DVE,
                    mybir.EngineType.Activation, mybir.EngineType.SP,
                    mybir.EngineType.Pool))
                loop_cm.__enter__()

            # prefetch the first GEMM group's k=0 operands ahead of phase 1
            # so the PE can start on (k=0, s=0) immediately
            xr0 = wp.tile([128, 1024], BF16, tag="xr", bufs=3, name="xr_0_0")
            nc.sync.dma_start(xr0[:], xt[0:128, :])
            wt0 = wp.tile([128, 512], BF16, tag="w", name="w1_0_0_0")
            nc.sync.dma_start(wt0[:], w1[0:128, 0:512])

            # ---- phase 1: layer-1 Chebyshev streams (T1..T4) ----
            st1 = [sp.tile([128, KT * 1024], BF16, tag=f"s{d}", name=f"l1s{d}")
                   for d in range(1, 5)]
            for k in range(KT):
                for n in range(2):
                    j = k * 2 + n
                    xbf = sc.tile([128, 512], BF16, tag="xbf",
                                  name=f"xbf_{j}")
                    nc.sync.dma_start(
                        xbf[:],
                        xt[k * 128:(k + 1) * 128, n * 512:(n + 1) * 512])
                    _emit_cheby_streams(nc, sc, xbf[:], st1, j)

            # layer-1 h accumulator / later the BN'd layer-2 base stream
            hh = sp.tile([128, MT1 * 1024], BF16, name="hh")
            s1p = [sp.tile([128, MT1], F32, name=f"s1p{n}") for n in range(2)]
            s2p = [sp.tile([128, MT1], F32, name=f"s2p{n}") for n in range(2)]
            scl = sp.tile([128, MT1], F32, name="scl")
            shf = sp.tile([128, MT1], F32, name="shf")

            def bn_coeffs(half, bnc_in, bnc_out):
                # combine n-halves, AllReduce, derive scale/shift — for one
                # half of the features (hm0 <= m < hm1)
                hm0 = half * (MT1 // 2)
                HM = MT1 // 2
                st = sp.tile([128, 2 * HM], F32, tag="stat_c",
                             name=f"st_{half}")
                nc.vector.tensor_add(st[:, 0:HM], s1p[0][:, hm0:hm0 + HM],
                                     s1p[1][:, hm0:hm0 + HM])
                nc.vector.tensor_add(st[:, HM:2 * HM], s2p[0][:, hm0:hm0 + HM],
                                     s2p[1][:, hm0:hm0 + HM])
                nc.sync.dma_start(bnc_in[:], st[:])
                if loop_k > 1 or no_cc:
                    nc.sync.dma_start(bnc_out[:], bnc_in[:])
                else:
                    nc.gpsimd.collective_compute(
                        "AllReduce", OP.add,
                        ins=[bnc_in.opt()], outs=[bnc_out.opt()],
                        replica_groups=[list(range(n_cores))])
                sto = sp.tile([128, 2 * HM], F32, tag="stat_o",
                              name=f"sto_{half}")
                nc.sync.dma_start(sto[:], bnc_out[:])
                meanv = sp.tile([128, HM], F32, tag="stat_m",
                                name=f"meanv_{half}")
                nc.vector.tensor_scalar_mul(meanv[:], sto[:, 0:HM],
                                            1.0 / b_total)
                e2 = sp.tile([128, HM], F32, tag="stat_e", name=f"e2_{half}")
                nc.vector.tensor_scalar_mul(e2[:], sto[:, HM:2 * HM],
                                            1.0 / b_total)
                mm2 = sp.tile([128, HM], F32, tag="stat_mm",
                              name=f"mm2_{half}")
                nc.vector.tensor_mul(mm2[:], meanv[:], meanv[:])
                varv = sp.tile([128, HM], F32, tag="stat_v",
                               name=f"varv_{half}")
                nc.vector.tensor_sub(varv[:], e2[:], mm2[:])
                nc.vector.tensor_scalar_add(varv[:], varv[:], BN_EPS)
                invv = sp.tile([128, HM], F32, tag="stat_i",
                               name=f"invv_{half}")
                nc.vector.reciprocal(invv[:], varv[:])
                istd = sp.tile([128, HM], F32, tag="stat_s",
                               name=f"istd_{half}")
                nc.scalar.sqrt(istd[:], invv[:])
                sl = scl[:, hm0:hm0 + HM]
                nc.vector.tensor_mul(sl, gams[:, hm0:hm0 + HM], istd[:])
                msc = sp.tile([128, HM], F32, tag="stat_ms",
                              name=f"msc_{half}")
                nc.vector.tensor_mul(msc[:], meanv[:], sl)
                nc.vector.tensor_sub(shf[:, hm0:hm0 + HM],
                                     bets[:, hm0:hm0 + HM], msc[:])

            bnc_in = [dp.tile([128, MT1], F32, name=f"bnc_in{h}")
                      for h in range(2)]
            bnc_out = [dp.tile([128, MT1], F32, addr_space="Shared",
                               name=f"bnc_out{h}") for h in range(2)]

            # ---- phase 2: layer-1 GEMM, groups of 4 m-tiles ----
            # (split-BN: stats for each feature half AllReduce as soon as
            # that half's groups finish, hiding AR latency under the GEMM)
            half0_done = False
            for g in range(MT1 // 4):
                acc = [ps.tile([128, 512], F32, tag="acc",
                               name=f"acc1_{g}_{i}") for i in range(8)]
                for k in range(KT):
                    if g == 0 and k == 0:
                        xr = xr0
                    else:
                        xr = wp.tile([128, 1024], BF16, tag="xr", bufs=3,
                                     name=f"xr_{g}_{k}")
                        nc.sync.dma_start(xr[:],
                                          xt[k * 128:(k + 1) * 128, :])
                    for s in range(NS):
                        if g == 0 and k == 0 and s == 0:
                            wt = wt0
                        else:
                            wt = wp.tile([128, 512], BF16, tag="w",
                                         name=f"w1_{g}_{k}_{s}")
                            nc.sync.dma_start(
                                wt[:],
                                w1[s * in_ + k * 128: s * in_ + (k + 1) * 128,
                                   g * 512:(g + 1) * 512])
                        for mi in range(4):
                            for n in range(2):
                                c0 = k * 1024 + n * 512
                                if s == 0:
                                    rhs = xr[:, n * 512:(n + 1) * 512]
                                else:
                                    rhs = st1[s - 1][:, c0:c0 + 512]
                                nc.tensor.matmul(
                                    acc[mi * 2 + n][:],
                                    wt[:, mi * 128:(mi + 1) * 128], rhs,
                                    start=(k == 0 and s == 0),
                                    stop=(k == KT - 1 and s == NS - 1))
                # finalize: bias add, cast to bf16 h, BN partial sums
                for mi in range(4):
                    m = g * 4 + mi
                    for n in range(2):
                        hs = hh[:, m * 1024 + n * 512: m * 1024 + (n + 1) * 512]
                        nc.scalar.activation(
                            hs, acc[mi * 2 + n][:], AF.Identity,
                            bias=b1s[:, m:m + 1], scale=1.0,
                            accum_out=s1p[n][:, m:m + 1])
                        sq = sc.tile([128, 512], BF16, tag="sq",
                                     name=f"sq_{g}_{mi}_{n}")
                        nc.scalar.activation(
                            sq[:], acc[mi * 2 + n][:], AF.Square,
                            bias=b1s[:, m:m + 1], scale=1.0,
                            accum_out=s2p[n][:, m:m + 1])
                if (g + 1) * 4 == MT1 // 2:
                    bn_coeffs(0, bnc_in[0], bnc_out[0])
                    half0_done = True

            # ---- phase 3: remaining BN stats ----
            if not half0_done:
                bn_coeffs(0, bnc_in[0], bnc_out[0])
            bn_coeffs(1, bnc_in[1], bnc_out[1])

            # ---- phase 4: BN apply (in place) + layer-2 streams ----
            st2 = [sp.tile([128, KT2 * 1024], BF16, tag=f"s{d}", name=f"l2s{d}")
                   for d in range(1, 5)]
            for j in range(MT1 * NCHUNK):
                m = j // 2
                c0 = j * 512
                hs = hh[:, c0:c0 + 512]
                nc.vector.tensor_scalar(
                    hs, hs, scl[:, m:m + 1], shf[:, m:m + 1], OP.mult, OP.add)
                _emit_cheby_streams(nc, sc, hs, st2, j)

            # ---- phase 5: layer-2 GEMM, groups of 4 m-tiles ----
            for g in range(MT2 // 4):
                acc = [ps.tile([128, 512], F32, tag="acc",
                               name=f"acc2_{g}_{i}") for i in range(8)]
                for k in range(KT2):
                    for s in range(NS):
                        wt = wp.tile([128, 512], BF16, tag="w",
                                     name=f"w2_{g}_{k}_{s}")
                        nc.sync.dma_start(
                            wt[:], w2[s * h_ + k * 128: s * h_ + (k + 1) * 128,
                                      g * 512:(g + 1) * 512])
                        for mi in range(4):
                            for n in range(2):
                                c0 = k * 1024 + n * 512
                                if s == 0:
                                    rhs = hh[:, c0:c0 + 512]
                                else:
                                    rhs = st2[s - 1][:, c0:c0 + 512]
                                nc.tensor.matmul(
                                    acc[mi * 2 + n][:],
                                    wt[:, mi * 128:(mi + 1) * 128], rhs,
                                    start=(k == 0 and s == 0),
                                    stop=(k == KT2 - 1 and s == NS - 1))
                for mi in range(4):
                    m = g * 4 + mi
                    for n in range(2):
                        osb = sc.tile([128, 512], F32, tag="osb",
                                      name=f"osb_{g}_{mi}_{n}")
                        nc.scalar.activation(
                            osb[:], acc[mi * 2 + n][:], AF.Identity,
                            bias=b2s[:, m:m + 1], scale=1.0)
                        nc.sync.dma_start(
                            out_t[m * 128:(m + 1) * 128,
                                  n * 512:(n + 1) * 512], osb[:])

            if loop_k > 1:
                loop_cm.__exit__(None, None, None)

    nc.compile()
    return nc


def _get_nc():
    global _CACHED_NC
    if _CACHED_NC is None:
        _CACHED_NC = _build()
    return _CACHED_NC


def kernel(x, w1_base, b1_base, c1, s1, bn_gamma, bn_beta,
           w2_base, b2_base, c2, s2, _trace=False, **_trace_kwargs):
    x = np.asarray(x, np.float32)
    bf = ml_dtypes.bfloat16

    # Fold T0=1 into the bias; fold the scaler into the coefficients.
    w1p = np.empty((NS * IN, H), bf)
    w1p[0:IN] = w1_base.T.astype(bf)
    for d in range(1, DEG + 1):
        w1p[d * IN:(d + 1) * IN] = (c1[:, :, d] * s1[None, :]).astype(bf)
    b1e = (b1_base + s1 * c1[:, :, 0].sum(axis=0)).astype(np.float32)

    w2p = np.empty((NS * H, OUT), bf)
    w2p[0:H] = w2_base.T.astype(bf)
    for d in range(1, DEG + 1):
        w2p[d * H:(d + 1) * H] = (c2[:, :, d] * s2[None, :]).astype(bf)
    b2e = (b2_base + s2 * c2[:, :, 0].sum(axis=0)).astype(np.float32)

    def fold_pm(v, mt):
        return np.ascontiguousarray(
            np.asarray(v, np.float32).reshape(mt, 128).T)

    mt1, mt2 = H // 128, OUT // 128
    common = {
        "w1": w1p, "b1": fold_pm(b1e, mt1),
        "gam": fold_pm(bn_gamma, mt1), "bet": fold_pm(bn_beta, mt1),
        "w2": w2p, "b2": fold_pm(b2e, mt2),
    }
    in_maps = []
    for c in range(N_CORES):
        xs = np.ascontiguousarray(x[c * BC:(c + 1) * BC].T.astype(bf))
        in_maps.append({"xt": xs, **common})

    nc = _get_nc()
    res = run_bass_kernel_spmd(nc, in_maps, core_ids=list(range(N_CORES)),
                               trace=_trace, **_trace_kwargs)
    out = np.concatenate(
        [res.results[c]["out_t"].T for c in range(N_CORES)], axis=0)
    if _trace:
        return np.ascontiguousarray(out, dtype=np.float32), res
    return np.ascontiguousarray(out, dtype=np.float32)
